# revision 34
# baseline (speedup 1.0000x reference)
"""MLA attention kernel (DeepSeek-style) for 8 Trainium2 NeuronCores.

Sharding: core = b*4 + g*2 + t over (batch b in {0,1}) x (head-group g in
{0,1}: 8 heads each) x (query-fold t in {0,1}).  Each core computes the
full latent pipeline for its batch, q/k/v + attention for its 8 heads and
its 1024 query tokens (two causally-folded 512-blocks), and a partial
output projection; the host sums the two head-group partials.

All tensors flow transposed ([feature-part, token-free]) so no on-chip
transposes are needed: the host supplies x^T per batch, and causal
structure is made SPMD-uniform by permuting the key order per core and
feeding full-block invalidation as per-partition bias columns consumed by
the Exp activation.  Matmul operands are fp16 (1 cyc/row on the PE; all
values are O(1) so fp16's 11-bit mantissa gives ~5e-4 rounding).

Perf structure:
- A-projections are feature-split across g-pair cores (which share the
  same token permutation): each core computes half the Q / KV latent
  chunks from host-sliced wq_a / wkv_a halves, then pairwise AllGather
  collectives (DRAM bounce, rank-major global order so the import is
  SPMD-uniform) restore the full latents on both cores.  This removes
  ~68us of duplicated PE work per core; collective ring latency (~44us)
  hides behind the independent KV chains / phase 3.
- x strips are processed [1,3,0,2] so query strips reuse resident x
  tiles; phase-3 rope chains are computed per head-PAIR (M=128 instead
  of two M=64 matmuls).
- Softmax denominators: a serial DVE add-chain collapses the exp tiles,
  one gpsimd partition_all_reduce broadcasts the key-sum (replacing 24
  PE ones-matmuls + broadcast matmul per head); un-normalized attn@v
  numerators are parked in SBUF by the scalar engine so PSUM banks
  recycle off that slow path.
"""

from contextlib import ExitStack

import numpy as np

import concourse.bacc as bacc
import concourse.bass as bass
import concourse.bass_isa as bass_isa
import concourse.tile as tile
from concourse import mybir
from concourse.bass_utils import run_bass_kernel_spmd

# Problem shapes (hardcoded per contest contract)
B, S, D = 2, 2048, 2048
H = 16
QL = 1536  # q lora rank
KVL = 512  # kv lora rank
NOPE = 128
ROPE = 64
VD = 128
QKD = NOPE + ROPE  # 192
EPS = 1e-6
SCALE = QKD ** (-0.5)

HPC = 8         # heads per core
NQ = 1024       # query tokens per core
P = 128
NEG = -30000.0  # additive mask value (exp -> 0)

F32 = mybir.dt.float32
F16 = mybir.dt.float16
EXP = mybir.ActivationFunctionType.Exp
RADD = bass_isa.ReduceOp.add

N_CORES = 8
SC_A = 8   # key 128-chunks for query block a
SC_B = 16  # key 128-chunks for query block b

ND = D // P        # 16
NRQ = QL // P      # 12
NRKV = KVL // P    # 4
HW = ROPE // 2     # 32
PAIRW = 2 * NOPE + 2 * ROPE  # 384: [nope_h0|nope_h1|rope_h0|rope_h1]
NQL = NRQ // 2     # 6: q latent chunks computed locally (g-pair split)
NKVL = NRKV // 2   # 2: kv latent chunks computed locally
# g-pair groups (same batch, same query-fold, different head group)
CC_GROUPS = [[0, 2], [1, 3], [4, 6], [5, 7]]

_CACHE = {}


def _rope(nc, pool, out_ap, ps, cos_ap, sin_ap, n):
    """rows 0:32 = even pair elems, 32:64 = odd.
    out[0:32] = e*cos - o*sin ; out[32:64] = e*sin + o*cos."""
    e = ps[0:HW, :]
    o = ps[HW:ROPE, :]
    t1 = pool.tile([HW, n], F32, tag="rp1", name="t1")
    nc.vector.tensor_mul(t1[:], e, cos_ap)
    t2 = pool.tile([HW, n], F32, tag="rp2", name="t2")
    nc.vector.tensor_mul(t2[:], o, sin_ap)
    nc.vector.tensor_sub(out_ap[0:HW, :], t1[:], t2[:])
    t3 = pool.tile([HW, n], F32, tag="rp3", name="t3")
    nc.vector.tensor_mul(t3[:], e, sin_ap)
    t4 = pool.tile([HW, n], F32, tag="rp4", name="t4")
    nc.vector.tensor_mul(t4[:], o, cos_ap)
    nc.vector.tensor_add(out_ap[HW:ROPE, :], t3[:], t4[:])


def build_nc():
    nc = bacc.Bacc("TRN2", target_bir_lowering=False, debug=False,
                   num_devices=N_CORES)

    def inp(name, shape, dt=F32):
        return nc.dram_tensor(name, shape, dt, kind="ExternalInput").ap()

    xT = inp("xT", [D, S], F16)
    wqa = inp("wq_a", [D, NQL * P], F16)
    wqb = inp("wq_b", [QL, (HPC // 2) * PAIRW], F16)
    wkva = inp("wkv_a", [D, NKVL * P + ROPE], F16)
    wkvbk = inp("wkv_b_k", [KVL, HPC * NOPE], F16)
    wkvbv = inp("wkv_b_v", [KVL, HPC * VD], F16)
    wo = inp("wo", [HPC * VD, D], F16)
    cosq = inp("cosq", [HW, NQ])
    sinq = inp("sinq", [HW, NQ])
    cosk = inp("cosk", [HW, S])
    sink = inp("sink", [HW, S])
    bias_a = inp("bias_a", [P, SC_A])
    bias_b = inp("bias_b", [P, SC_B])
    out = nc.dram_tensor("out", [NQ, D], F32, kind="ExternalOutput").ap()

    with tile.TileContext(nc) as tc, ExitStack() as ctx, \
            nc.allow_low_precision(reason="fp16 matmul pipeline"):
        const = ctx.enter_context(tc.tile_pool(name="const", bufs=1))
        ones_cf = const.tile([P, 1], F32, tag="ones_cf")
        nc.vector.memset(ones_cf[:], 1.0)
        ones_c = const.tile([P, 1], F16, tag="ones_c")
        nc.vector.tensor_copy(ones_c[:], ones_cf[:])
        ones_rf = const.tile([1, P], F32, tag="ones_rf")
        nc.vector.memset(ones_rf[:], 1.0)
        ones_r = const.tile([1, P], F16, tag="ones_r")
        nc.vector.tensor_copy(ones_r[:], ones_rf[:])
        # multiplicative staircase masks (1 keep / 0 drop), applied post-exp
        stairs = []
        for j in range(4):
            st = const.tile([P, 512], F16, tag=f"stair{j}", name=f"st{j}")
            nc.vector.memset(st[:], 1.0)
            # keep 1 where f - p - 128j >= 0 else 0
            nc.gpsimd.affine_select(
                out=st[:], in_=st[:], compare_op=mybir.AluOpType.is_ge,
                fill=0.0, base=-128 * j, pattern=[[1, 512]],
                channel_multiplier=-1)
            stairs.append(st)
        bias_a_sb = const.tile([P, SC_A], F32, tag="bias_a")
        nc.sync.dma_start(bias_a_sb[:], bias_a[:])
        bias_b_sb = const.tile([P, SC_B], F32, tag="bias_b")
        nc.sync.dma_start(bias_b_sb[:], bias_b[:])
        eps_t = const.tile([P, 1], F32, tag="eps")
        nc.vector.memset(eps_t[:], EPS)

        # persistent: kv latents + k_pe as two zero-padded K=128 variants
        latA = ctx.enter_context(tc.tile_pool(name="latA", bufs=1))
        kvT = [latA.tile([P, S], F16, tag=f"kvT{i}", name=f"kvT{i}")
               for i in range(NRKV)]
        kpe_e = latA.tile([P, S], F16, tag="kpe_e")
        kpe_o = latA.tile([P, S], F16, tag="kpe_o")
        nc.vector.memset(kpe_e[ROPE:P, :], 0.0)
        nc.vector.memset(kpe_o[0:ROPE, :], 0.0)
        kpez = (kpe_e, kpe_o)

        # packed cq latent: 24 [128,512] slices (rc, tbq) in 8 tiles;
        # reused as oTn after phase 3
        latQ = ctx.enter_context(tc.tile_pool(name="latQ", bufs=1))
        cqPk = [latQ.tile([P, 1536], F16, tag=f"cqPk{i}", name=f"cqPk{i}")
                for i in range(8)]

        def cq_slice(rc, tbq):
            idx = rc * 2 + tbq
            t, c = idx // 3, (idx % 3) * 512
            return cqPk[t][:, c:c + 512]

        oTn = [cqPk[h][:, 0:NQ] for h in range(HPC)]

        ps_main = ctx.enter_context(
            tc.tile_pool(name="ps_main", bufs=4, space="PSUM"))

        def mm_chain(ps_ap, pairs):
            n = len(pairs)
            for i, (lh, rh) in enumerate(pairs):
                nc.tensor.matmul(ps_ap, lh, rh,
                                 start=(i == 0), stop=(i == n - 1))

        ps_x_ctx = ExitStack()
        ps_x = ps_x_ctx.enter_context(
            tc.tile_pool(name="ps_x", bufs=2, space="PSUM"))

        sqp = ctx.enter_context(tc.tile_pool(name="sq", bufs=3))

        def normalize(which, tbs):
            nrc, nfeat = ((NRKV, KVL) if which == 0 else (NRQ, QL))

            def sl_of(oc, tb):
                if which == 0:
                    return kvT[oc][:, tb * 512:(tb + 1) * 512]
                return cq_slice(oc, tb)
            for tb in tbs:
                pss = ps_x.tile([1, 512], F32, tag="pss", name="pss")
                for oc in range(nrc):
                    sq = sqp.tile([P, 512], F16, tag="sq", name="sq")
                    nc.scalar.activation(
                        sq[:], sl_of(oc, tb),
                        mybir.ActivationFunctionType.Square)
                    nc.tensor.matmul(pss[:], ones_c[:], sq[:],
                                     start=(oc == 0), stop=(oc == nrc - 1))
                sd = sqp.tile([1, 512], F16, tag="sd", name="sd")
                nc.scalar.activation(
                    sd[:], pss[:], mybir.ActivationFunctionType.Sqrt,
                    bias=eps_t[0:1, :], scale=1.0 / nfeat)
                psb = ps_main.tile([P, 512], F32, tag="ps", name="psb")
                nc.tensor.matmul(psb[:], ones_r[:], sd[:],
                                 start=True, stop=True)
                rb = sqp.tile([P, 512], F32, tag="rb", name="rb")
                nc.vector.reciprocal_approx_fast(rb[:], psb[:])
                for oc in range(nrc):
                    nc.vector.tensor_mul(sl_of(oc, tb), sl_of(oc, tb),
                                         rb[:])

        # ---------- Phase 1: A-projections, feature-split across g-pairs --
        # Each core computes half the Q latent chunks (host supplies its
        # wq_a half) and half the KV latent chunks + the full rope columns;
        # pairwise AllGather via DRAM bounce restores the full latents in
        # GLOBAL chunk order on both cores (rank-major gather order is core
        # independent, so the import is SPMD-uniform).
        dram = ctx.enter_context(tc.tile_pool(name="dram", bufs=1,
                                              space="DRAM"))
        q_bin = dram.tile([P, NQL * 2 * 512], F16, tag="q_bin",
                          name="q_bin")
        q_bout = dram.tile([2 * P, NQL * 2 * 512], F16, tag="q_bout",
                           name="q_bout")
        kv_bin = dram.tile([P, NKVL * S], F16, tag="kv_bin")
        kv_bout = dram.tile([2 * P, NKVL * S], F16, tag="kv_bout")

        with ExitStack() as p1:
            tabk = p1.enter_context(tc.tile_pool(name="tabk", bufs=2))
            ropep = p1.enter_context(tc.tile_pool(name="ropep", bufs=2))
            wkva_p = p1.enter_context(tc.tile_pool(name="wkva", bufs=1))
            wqa_p = p1.enter_context(tc.tile_pool(name="wqa", bufs=1))
            xt_p = p1.enter_context(tc.tile_pool(name="xt", bufs=2))
            stg_p = p1.enter_context(tc.tile_pool(name="stg", bufs=4))

            def load_strip(tb):
                sl = slice(tb * 512, (tb + 1) * 512)
                xts = []
                for dc in range(ND):
                    xt = xt_p.tile([P, 512], F16, tag=f"xt{dc}",
                                   name=f"xt{tb}_{dc}")
                    nc.sync.dma_start(xt[:], xT[dc * P:(dc + 1) * P, sl])
                    xts.append(xt)
                return xts

            # DMA priority: x strip 1, wq_a half, x strip 3, wkv_a
            xts1 = load_strip(1)
            wqa_sb = []
            for dc in range(ND):
                wt = wqa_p.tile([P, NQL * P], F16, tag=f"wqa{dc}",
                                name=f"wqa{dc}")
                nc.sync.dma_start(wt[:], wqa[dc * P:(dc + 1) * P, :])
                wqa_sb.append(wt)
            xts3 = load_strip(3)
            wkva_sb = []
            for dc in range(ND):
                wt = wkva_p.tile([P, NKVL * P + ROPE], F16, tag=f"wkva{dc}",
                                 name=f"wkva{dc}")
                nc.sync.dma_start(wt[:], wkva[dc * P:(dc + 1) * P, :])
                wkva_sb.append(wt)

            def q_chains_local(tbq, xts):
                for oc in range(NQL):
                    pool = ps_main if oc % 2 == 0 else ps_x
                    ps = pool.tile([P, 512], F32, tag="ps", name="ps1b")
                    mm_chain(ps[:], [
                        (wqa_sb[dc][:, oc * P:(oc + 1) * P], xts[dc][:])
                        for dc in range(ND)])
                    st = stg_p.tile([P, 512], F16, tag=f"qs{oc % 2}",
                                    name="qstg")
                    if oc % 2 == 0:
                        nc.vector.tensor_copy(st[:], ps[:])
                    else:
                        nc.scalar.copy(st[:], ps[:])
                    nc.sync.dma_start(
                        q_bin[:, (oc * 2 + tbq) * 512:
                              (oc * 2 + tbq + 1) * 512], st[:])

            def kv_chains_local(tb, xts):
                sl = slice(tb * 512, (tb + 1) * 512)
                for oc in range(NKVL):
                    pool = ps_main if oc % 2 == 0 else ps_x
                    ps = pool.tile([P, 512], F32, tag="ps", name="ps1")
                    mm_chain(ps[:], [
                        (wkva_sb[dc][:, oc * P:(oc + 1) * P], xts[dc][:])
                        for dc in range(ND)])
                    st = stg_p.tile([P, 512], F16, tag=f"ks{oc % 2}",
                                    name="kstg")
                    if oc % 2 == 0:
                        nc.vector.tensor_copy(st[:], ps[:])
                    else:
                        nc.scalar.copy(st[:], ps[:])
                    nc.sync.dma_start(
                        kv_bin[:, oc * S + tb * 512:oc * S + tb * 512 + 512],
                        st[:])
                psp = ps_main.tile([ROPE, 512], F32, tag="ps", name="ps1p")
                mm_chain(psp[:], [
                    (wkva_sb[dc][:, NKVL * P:NKVL * P + ROPE], xts[dc][:])
                    for dc in range(ND)])
                ck = tabk.tile([HW, 512], F32, tag="cosk", name="ck")
                nc.sync.dma_start(ck[:], cosk[:, sl])
                sk = tabk.tile([HW, 512], F32, tag="sink", name="sk")
                nc.sync.dma_start(sk[:], sink[:, sl])
                _rope(nc, ropep, kpe_e[0:ROPE, sl], psp, ck[:], sk[:], 512)
                nc.sync.dma_start(kpe_o[ROPE:P, sl], kpe_e[0:ROPE, sl])

            # single collective: the NRT ring costs ~44us nearly payload-
            # independent, so two half-size gathers would serialize and
            # finish later than one merged gather
            q_chains_local(0, xts1)
            q_chains_local(1, xts3)
            nc.gpsimd.collective_compute(
                "AllGather", mybir.AluOpType.bypass,
                replica_groups=CC_GROUPS,
                ins=[q_bin.opt()], outs=[q_bout.opt()])
            for rcg in range(NRQ):
                r, ocl = divmod(rcg, NQL)
                for tbq in range(2):
                    nc.sync.dma_start(
                        cq_slice(rcg, tbq),
                        q_bout[r * P:(r + 1) * P,
                               (ocl * 2 + tbq) * 512:
                               (ocl * 2 + tbq + 1) * 512])

            # normalize is emitted after ALL kv chains and the kv gather
            # issue: its ones-matmuls wait on the q AllGather, and placing
            # them mid-phase-1 would stall the in-order PE queue with kv
            # work still pending (and delay the AG(kv) trigger behind it)
            kv_chains_local(1, xts1)
            kv_chains_local(3, xts3)
            xts0 = load_strip(0)
            kv_chains_local(0, xts0)
            xts2 = load_strip(2)
            kv_chains_local(2, xts2)
            nc.gpsimd.collective_compute(
                "AllGather", mybir.AluOpType.bypass,
                replica_groups=CC_GROUPS,
                ins=[kv_bin.opt()], outs=[kv_bout.opt()])
            normalize(1, [0])
            normalize(1, [1])
            for j in range(NRKV):
                r, ocl = divmod(j, NKVL)
                nc.sync.dma_start(
                    kvT[j][:],
                    kv_bout[r * P:(r + 1) * P, ocl * S:(ocl + 1) * S])

        # ---------- Phase 3: qT for all heads (rope per head-pair) --------
        latQT = ctx.enter_context(tc.tile_pool(name="latQT", bufs=1))
        qTn = [latQT.tile([P, NQ], F16, tag=f"qTn{h}", name=f"qTn{h}")
               for h in range(HPC)]
        qTpk = [latQT.tile([P, NQ], F16, tag=f"qTpk{i}", name=f"qTpk{i}")
                for i in range(HPC // 2)]
        with ExitStack() as p3:
            tabq = p3.enter_context(tc.tile_pool(name="tabq", bufs=1))
            cq_sb = tabq.tile([HW, NQ], F32, tag="cosq")
            nc.sync.dma_start(cq_sb[:], cosq[:])
            sq_sb = tabq.tile([HW, NQ], F32, tag="sinq")
            nc.sync.dma_start(sq_sb[:], sinq[:])
            ropep3 = p3.enter_context(tc.tile_pool(name="ropep3", bufs=2))
            wqb_p = p3.enter_context(tc.tile_pool(name="wqb", bufs=3))
            for pp in range(HPC // 2):
                if pp == 3:
                    # kv latents arrived from the pair AllGather; normalize
                    # them here so the scaling overlaps the last q pair
                    normalize(0, [0, 1, 2, 3])
                wqb_sb = []
                for rc in range(NRQ):
                    wt = wqb_p.tile([P, PAIRW], F16, tag=f"wqb{rc}",
                                    name=f"wqb{rc}")
                    nc.sync.dma_start(
                        wt[:], wqb[rc * P:(rc + 1) * P,
                                   pp * PAIRW:(pp + 1) * PAIRW])
                    wqb_sb.append(wt)
                for tbq in range(2):
                    sl = slice(tbq * 512, (tbq + 1) * 512)
                    for hh in range(2):
                        h = 2 * pp + hh
                        ps = ps_main.tile([P, 512], F32, tag="ps", name="ps3")
                        mm_chain(ps[:], [
                            (wqb_sb[rc][:, hh * NOPE:(hh + 1) * NOPE],
                             cq_slice(rc, tbq))
                            for rc in range(NRQ)])
                        if hh == 0:
                            nc.vector.tensor_copy(qTn[h][:, sl], ps[:])
                        else:
                            nc.scalar.copy(qTn[h][:, sl], ps[:])
                    psp = ps_x.tile([P, 512], F32, tag="ps", name="ps3p")
                    mm_chain(psp[:], [
                        (wqb_sb[rc][:, 2 * NOPE:PAIRW], cq_slice(rc, tbq))
                        for rc in range(NRQ)])
                    _rope(nc, ropep3, qTpk[pp][0:ROPE, sl], psp[0:ROPE, :],
                          cq_sb[:, sl], sq_sb[:, sl], 512)
                    _rope(nc, ropep3, qTpk[pp][ROPE:P, sl], psp[ROPE:P, :],
                          cq_sb[:, sl], sq_sb[:, sl], 512)

        ps_x_ctx.close()

        # ---------- Phase 4: attention per head-pair ----------
        wo_p = ctx.enter_context(tc.tile_pool(name="wo", bufs=1))
        wo_sb = []
        with ExitStack() as p4:
            kt_p = p4.enter_context(tc.tile_pool(name="kt", bufs=4))
            v_p = p4.enter_context(tc.tile_pool(name="v", bufs=2))
            wk_p = p4.enter_context(tc.tile_pool(name="wkvb", bufs=2))
            work = p4.enter_context(tc.tile_pool(name="work", bufs=2))
            sum_p = p4.enter_context(tc.tile_pool(name="sum", bufs=2))
            red_p = p4.enter_context(tc.tile_pool(name="red", bufs=2))
            ou_p = p4.enter_context(tc.tile_pool(name="ou", bufs=2))
            ptp = p4.enter_context(tc.tile_pool(name="ptp", bufs=28))
            ps_o = p4.enter_context(
                tc.tile_pool(name="ps_o", bufs=4, space="PSUM"))
            for hp in range(HPC // 2):
                if hp == HPC // 2 - 1:
                    # wo is needed only in phase 5; issue its 4MB of DMA
                    # during the last pair when the queues are idle
                    for h in range(HPC):
                        wt = wo_p.tile([P, D], F16, tag=f"wo{h}",
                                       name=f"wo{h}")
                        nc.sync.dma_start(wt[:], wo[h * P:(h + 1) * P, :])
                        wo_sb.append(wt)
                heads = (2 * hp, 2 * hp + 1)
                kT = {}
                for h in heads:
                    wk_sb = []
                    for rc in range(NRKV):
                        wt = wk_p.tile([P, NOPE], F16, tag=f"wkvbk{rc}",
                                       name=f"wkk{rc}")
                        nc.sync.dma_start(
                            wt[:], wkvbk[rc * P:(rc + 1) * P,
                                         h * NOPE:(h + 1) * NOPE])
                        wk_sb.append(wt)
                    kt = kt_p.tile([P, S], F16, tag="kt", name=f"kt{h}")
                    for tb in range(4):
                        sl = slice(tb * 512, (tb + 1) * 512)
                        ps = ps_main.tile([P, 512], F32, tag="ps", name="ps4k")
                        mm_chain(ps[:], [(wk_sb[rc][:], kvT[rc][:, sl])
                                         for rc in range(NRKV)])
                        if tb % 2 == 0:
                            nc.vector.tensor_copy(kt[:, sl], ps[:])
                        else:
                            nc.scalar.copy(kt[:, sl], ps[:])
                    kT[h] = kt
                wv_sb = []
                for rc in range(NRKV):
                    wt = wk_p.tile([P, 2 * VD], F16, tag=f"wkvbv{rc}",
                                   name=f"wkv{rc}")
                    nc.sync.dma_start(
                        wt[:], wkvbv[rc * P:(rc + 1) * P,
                                     heads[0] * VD:(heads[0] + 2) * VD])
                    wv_sb.append(wt)
                vt = v_p.tile([P, 16 * 2 * VD], F16, tag="vt", name="vt")
                for tk in range(16):
                    ps = ps_main.tile([P, 2 * VD], F32, tag="ps", name="ps4v")
                    mm_chain(ps[:], [
                        (kvT[rc][:, tk * P:(tk + 1) * P], wv_sb[rc][:])
                        for rc in range(NRKV)])
                    if tk % 2 == 0:
                        nc.vector.tensor_copy(
                            vt[:, tk * 2 * VD:(tk + 1) * 2 * VD], ps[:])
                    else:
                        nc.scalar.copy(
                            vt[:, tk * 2 * VD:(tk + 1) * 2 * VD], ps[:])

                for h in heads:
                    hv = h % 2
                    pts = {0: [], 1: []}

                    # diagonal chunks (staircase jd>=1) have query columns
                    # [0:128*jd] entirely masked: trim the scores matmuls
                    # and exp to [off:512] and zero-fill the dead columns
                    def off_of(sc, qb):
                        nsc = SC_A if qb == 0 else SC_B
                        jd = sc - (nsc - 4)
                        return (P * jd if jd >= 1 else 0), jd

                    for sc in range(SC_B):
                        # both query blocks share each stationary load
                        sps = {}
                        for qb in ((0, 1) if sc < SC_A else (1,)):
                            off, _ = off_of(sc, qb)
                            sps[qb] = ps_main.tile([P, 512], F32, tag="ps",
                                                   name="ps4s")
                            nc.tensor.matmul(
                                sps[qb][:, off:512],
                                kT[h][:, sc * P:(sc + 1) * P],
                                qTn[h][:, qb * 512 + off:qb * 512 + 512],
                                start=True, stop=False)
                        for qb in sps:
                            off, _ = off_of(sc, qb)
                            nc.tensor.matmul(
                                sps[qb][:, off:512],
                                kpez[hv][:, sc * P:(sc + 1) * P],
                                qTpk[h // 2][:, qb * 512 + off:
                                              qb * 512 + 512],
                                start=False, stop=True)
                        for qb in sps:
                            nsc = SC_A if qb == 0 else SC_B
                            bias_sb = bias_a_sb if qb == 0 else bias_b_sb
                            pt = ptp.tile([P, 512], F16, tag="pt", name="pt")
                            off, jd = off_of(sc, qb)
                            if jd >= 0:
                                # [0:off] is never read downstream (attn@v
                                # and the add-chain are trimmed too)
                                nc.scalar.activation(pt[:, off:512],
                                                     sps[qb][:, off:512],
                                                     EXP)
                                nc.vector.tensor_mul(pt[:, off:512],
                                                     pt[:, off:512],
                                                     stairs[jd][:, off:512])
                            else:
                                nc.scalar.activation(
                                    pt[:], sps[qb][:], EXP,
                                    bias=bias_sb[:, sc:sc + 1])
                            pts[qb].append(pt)
                    oT = {qb: ps_o.tile([P, 512], F32, tag="oT",
                                        name=f"oT{qb}") for qb in (0, 1)}
                    for sc in range(SC_B):
                        for qb in ((0, 1) if sc < SC_A else (1,)):
                            nsc = SC_A if qb == 0 else SC_B
                            off, _ = off_of(sc, qb)
                            nc.tensor.matmul(
                                oT[qb][:, off:512],
                                vt[:, sc * 2 * VD + hv * VD:
                                   sc * 2 * VD + (hv + 1) * VD],
                                pts[qb][sc][:, off:512], start=(sc == 0),
                                stop=(sc == nsc - 1))
                    # park the un-normalized numerators in SBUF right away
                    # so the PSUM banks recycle off the (slow) denominator
                    # path
                    ou = {}
                    for qb in (0, 1):
                        ou[qb] = ou_p.tile([P, 512], F16, tag=f"ou{qb}",
                                           name=f"ou{h}_{qb}")
                        # scalar engine: the DVE is busy with the add-chain
                        # here and would delay the PSUM-bank release
                        nc.scalar.copy(ou[qb][:], oT[qb][:])
                    # softmax denominators: serial DVE add-chain collapses
                    # the exp tiles; gpsimd partition-reduce broadcasts the
                    # key-sum; scale numerators by 1/sum
                    for qb in (0, 1):
                        lst = pts[qb]
                        sm = sum_p.tile([P, 512], F16, tag=f"S{qb}",
                                        name=f"S{h}_{qb}")
                        nc.vector.tensor_add(sm[:], lst[0][:], lst[1][:])
                        for sc2, ptt in enumerate(lst[2:], start=2):
                            off, _ = off_of(sc2, qb)
                            if off:
                                nc.vector.tensor_add(sm[:, off:512],
                                                     sm[:, off:512],
                                                     ptt[:, off:512])
                            else:
                                nc.vector.tensor_add(sm[:], sm[:], ptt[:])
                        dd = red_p.tile([P, 512], F32, tag=f"D{qb}",
                                        name=f"D{h}_{qb}")
                        nc.gpsimd.partition_all_reduce(dd[:], sm[:],
                                                       channels=P,
                                                       reduce_op=RADD)
                        rb = work.tile([P, 512], F32, tag="rb", name="rb")
                        nc.vector.reciprocal_approx_fast(rb[:], dd[:])
                        nc.vector.tensor_mul(
                            oTn[h][:, qb * 512:(qb + 1) * 512],
                            ou[qb][:], rb[:])

        # ---------- Phase 5: output projection (wo aliases qTn/kvT) -------
        with ExitStack() as p5:
            os_p = p5.enter_context(tc.tile_pool(name="os", bufs=4))
            for tk in range(NQ // P):
                for dcb in range(4):
                    ps = ps_main.tile([P, 512], F32, tag="ps", name="ps5")
                    for h in range(HPC):
                        rh = wo_sb[h][:, dcb * 512:(dcb + 1) * 512]
                        nc.tensor.matmul(
                            ps[:], oTn[h][:, tk * P:(tk + 1) * P], rh,
                            start=(h == 0), stop=(h == HPC - 1))
                    ot = os_p.tile([P, 512], F32, tag="ot", name="ot")
                    # DVE: ScalarE-rate copies are 719ns here vs ~267ns on
                    # the (idle-in-phase-5) vector engine
                    nc.vector.tensor_copy(ot[:], ps[:])
                    nc.sync.dma_start(
                        out[tk * P:(tk + 1) * P,
                            dcb * 512:(dcb + 1) * 512], ot[:])

    nc.compile()
    return nc


def _prep_inputs(x, freqs_cis, wq_a, q_norm_w, wq_b, wkv_a, kv_norm_w,
                 wkv_b, wo):
    """Host-side shard prep. Returns (in_maps, meta) for 8 cores."""
    x = np.asarray(x, np.float32)
    freqs_cis = np.asarray(freqs_cis, np.float32)
    wq_a = np.asarray(wq_a, np.float32)
    q_norm_w = np.asarray(q_norm_w, np.float32)
    wq_b = np.asarray(wq_b, np.float32)
    wkv_a = np.asarray(wkv_a, np.float32)
    kv_norm_w = np.asarray(kv_norm_w, np.float32)
    wkv_b = np.asarray(wkv_b, np.float32)
    wo = np.asarray(wo, np.float32)

    f16 = np.float16
    # de-interleave perm for rope pairs: [e0..e31, o0..o31]
    perm = np.concatenate([np.arange(0, ROPE, 2), np.arange(1, ROPE, 2)])

    wqb = (wq_b * q_norm_w[:, None] * SCALE).reshape(QL, H, QKD)
    wqb_n = wqb[:, :, :NOPE]
    wqb_r = wqb[:, :, NOPE:][:, :, perm]
    # per head-pair packing: [nope_h0 | nope_h1 | rope_h0 | rope_h1]
    wqb_pk = np.concatenate(
        [wqb_n.reshape(QL, H // 2, 2 * NOPE),
         wqb_r.reshape(QL, H // 2, 2 * ROPE)], axis=2).astype(f16)

    # per-g-half wkv_a: my 2 latent chunks + the full (perm'd) rope cols
    wkva_g = [np.ascontiguousarray(np.concatenate(
        [wkv_a[:, g * NKVL * P:(g + 1) * NKVL * P],
         wkv_a[:, KVL:][:, perm]], axis=1).astype(f16)) for g in range(2)]

    wkvb = (wkv_b * kv_norm_w[:, None]).reshape(KVL, H, NOPE + VD).astype(f16)
    wkvb_k = wkvb[:, :, :NOPE]
    wkvb_v = wkvb[:, :, NOPE:]

    wqa_g = [np.ascontiguousarray(
        wq_a[:, g * NQL * P:(g + 1) * NQL * P].astype(f16))
        for g in range(2)]

    cos_t = np.ascontiguousarray(freqs_cis[:, :, 0].T)  # [32, S]
    sin_t = np.ascontiguousarray(freqs_cis[:, :, 1].T)

    sig0 = np.arange(S)
    sig1 = np.concatenate([sig0[512:1024], sig0[0:512],
                           sig0[1536:2048], sig0[1024:1536]])
    qpos = {0: np.concatenate([sig0[512:1024], sig0[1536:2048]]),
            1: np.concatenate([sig0[0:512], sig0[1024:1536]])}

    bias_a0 = np.zeros((P, SC_A), np.float32)
    bias_b0 = np.zeros((P, SC_B), np.float32)
    bias_a1 = np.zeros((P, SC_A), np.float32)
    bias_a1[:, 0:4] = NEG
    bias_b1 = np.zeros((P, SC_B), np.float32)
    bias_b1[:, 8:12] = NEG

    in_maps = []
    meta = []
    for c in range(N_CORES):
        b, g, t = c // 4, (c // 2) % 2, c % 2
        sig = sig0 if t == 0 else sig1
        hs = slice(g * HPC, (g + 1) * HPC)
        ps = slice(g * (HPC // 2), (g + 1) * (HPC // 2))
        m = {
            "xT": np.ascontiguousarray(x[b].T[:, sig].astype(f16)),
            "wq_a": wqa_g[g],
            "wq_b": np.ascontiguousarray(
                wqb_pk[:, ps, :].reshape(QL, (HPC // 2) * PAIRW)),
            "wkv_a": wkva_g[g],
            "wkv_b_k": np.ascontiguousarray(
                wkvb_k[:, hs, :].reshape(KVL, HPC * NOPE)),
            "wkv_b_v": np.ascontiguousarray(
                wkvb_v[:, hs, :].reshape(KVL, HPC * VD)),
            "wo": np.ascontiguousarray(
                wo[g * HPC * VD:(g + 1) * HPC * VD, :].astype(f16)),
            "cosq": np.ascontiguousarray(cos_t[:, qpos[t]]),
            "sinq": np.ascontiguousarray(sin_t[:, qpos[t]]),
            "cosk": np.ascontiguousarray(cos_t[:, sig]),
            "sink": np.ascontiguousarray(sin_t[:, sig]),
            "bias_a": bias_a0 if t == 0 else bias_a1,
            "bias_b": bias_b0 if t == 0 else bias_b1,
        }
        in_maps.append(m)
        meta.append((b, g, t))
    return in_maps, meta


def kernel(**inputs):
    in_maps, meta = _prep_inputs(**inputs)
    if "nc" not in _CACHE:
        _CACHE["nc"] = build_nc()
    nc = _CACHE["nc"]
    res = run_bass_kernel_spmd(nc, in_maps, core_ids=list(range(N_CORES)),
                               **_CACHE.get("run_kwargs", {}))
    _CACHE["last_result"] = res
    out = np.zeros((B, S, D), np.float32)
    for c in range(N_CORES):
        b, g, t = meta[c]
        part = res.results[c]["out"]  # [1024, 2048]
        if t == 0:
            out[b, 512:1024] += part[:512]
            out[b, 1536:2048] += part[512:]
        else:
            out[b, 0:512] += part[:512]
            out[b, 1024:1536] += part[512:]
    return out

